# revision 26
# baseline (speedup 1.0000x reference)
"""Trainium2 Bass kernel for BaseWindowAttention.

Problem (hardcoded): x [2,8,64,64,256] f32, w_qkv [256,768], w_out [256,256],
b_out [256], pos_embedding [15,15], window_size 8, heads 8, dim_head 32.

Strategy:
- Data parallel: 16 (b,l) images over 8 cores -> 2 images/core.
- Host: window-major channel-first bf16 transpose of x; fold softmax scale
  into w_q; precompute exp(bias) 2-window super-tile (off-diagonal zeros kill
  cross-window attention terms); replicate b_out across 128 partitions.
- Device per core, software-pipelined over strips of 512 tokens (8 windows):
  slot s interleaves projections+dots+exp of strip s with the attention tail
  (AV, normalize, transpose, out-projection) of strips s-2/s-3, so the PE
  stays back-to-back busy and the HAM clock gate holds 2.4 GHz.
- Engine split (GPSIMD cannot touch PSUM): exp on ACT; exp(bias) multiply
  split Pool/DVE (flat [128,1024] tiles keep 1-D free APs); PSUM->SBUF casts
  split ACT/DVE; normalize = DVE reciprocal of the fused denominator column
  + broadcast multiply; final bias add as DVE tensor_add with b_out
  replicated across partitions (no rank-1 PE matmuls).
- PSUM (8 banks): qkps f32 [128,512] x1 + dots f32 [128,1024] x2(x2 banks)
  + small (av/out/v/transpose, 1 bank each) x3. The slot-level issue order
  interleaves qk-projection sub-blocks with the shifted tail so each tag's
  round-robin lands on banks whose previous tenant has drained.
"""

import os
import sys
import numpy as np

sys.path.insert(0, "/opt/trn_rl_repo")
os.environ.setdefault("JAX_PLATFORMS", "")

import ml_dtypes

BF16 = ml_dtypes.bfloat16

B, L, H, W, C = 2, 8, 64, 64, 256
WS = 8
NHEADS = 8
CH = 32
N_CORES = 8
IMG = B * L                 # 16 images
IMG_PER_CORE = IMG // N_CORES
T_IMG = H * W               # 4096 tokens per image
STRIP = 512                 # tokens per strip (8 windows)
N_STRIPS = T_IMG // STRIP   # 8
NWP = STRIP // 128          # 4 window pairs per strip
NSLOT = IMG_PER_CORE * N_STRIPS  # 16 strips per core

_CACHE = {}


def _relative_indices(ws):
    idx = np.array([[i, j] for i in range(ws) for j in range(ws)])
    rel = idx[None, :, :] - idx[:, None, :] + ws - 1
    return rel


def _env(name, default):
    return os.environ.get(name, default)


def _build_kernel():
    import concourse.bass as bass
    import concourse.mybir as mybir
    import concourse.tile as tile
    from concourse import bacc
    from contextlib import ExitStack

    dt = mybir.dt
    nc = bacc.Bacc("TRN2", target_bir_lowering=False, debug=False)

    xT = nc.dram_tensor("xT", [IMG_PER_CORE, C, T_IMG], dt.bfloat16,
                        kind="ExternalInput").ap()
    wqk = nc.dram_tensor("wqk", [C, 512], dt.bfloat16, kind="ExternalInput").ap()
    wv = nc.dram_tensor("wv", [C, C], dt.bfloat16, kind="ExternalInput").ap()
    wout = nc.dram_tensor("wout", [C, C], dt.bfloat16, kind="ExternalInput").ap()
    bout128 = nc.dram_tensor("bout128", [128, C], dt.bfloat16,
                             kind="ExternalInput").ap()
    ebrep = nc.dram_tensor("ebrep", [128, 2048], dt.bfloat16,
                           kind="ExternalInput").ap()
    ident = nc.dram_tensor("ident", [128, 128], dt.bfloat16,
                           kind="ExternalInput").ap()
    out = nc.dram_tensor("out", [IMG_PER_CORE, T_IMG, C], dt.bfloat16,
                         kind="ExternalOutput").ap()

    EXP = mybir.ActivationFunctionType.Exp
    DIV = mybir.AluOpType.divide

    # PSUM bank budget (8): qkps f32 1*1 + dps f32 2*2 + small 3*1 = 8
    QKPS_BUFS = int(_env("QKPS_BUFS", "1"))
    DPS_BUFS = int(_env("DPS_BUFS", "2"))
    SMALL_BUFS = int(_env("SMALL_BUFS", "3"))
    QK_ENG = _env("QK_ENG", "aadd")     # per ot: a=ACT, d=DVE
    V_ENG = _env("V_ENG", "ad")         # per v pair
    EB_ENG = _env("EB_ENG", "ppdd")     # per dots tile: p=Pool, d=DVE
    NORM_DIV = _env("NORM_DIV", "0") == "1"

    def copy_eng(ch, out_t, in_t):
        if ch == "a":
            nc.scalar.copy(out_t, in_t)
        else:
            nc.vector.tensor_copy(out_t, in_t)

    with tile.TileContext(nc) as tc:
        with ExitStack() as ctx:
            consts = ctx.enter_context(tc.tile_pool(name="consts", bufs=1))
            xp = ctx.enter_context(tc.tile_pool(name="xp", bufs=4))
            qkp = ctx.enter_context(tc.tile_pool(name="qkp", bufs=10))
            vp = ctx.enter_context(tc.tile_pool(name="vp", bufs=8))
            ep = ctx.enter_context(tc.tile_pool(name="ep", bufs=8))
            anp = ctx.enter_context(tc.tile_pool(name="anp", bufs=6))
            aotp = ctx.enter_context(tc.tile_pool(name="aotp", bufs=6))
            fop = ctx.enter_context(tc.tile_pool(name="fop", bufs=6))
            psp = ctx.enter_context(tc.tile_pool(name="psp", bufs=1, space="PSUM"))

            # ---- constants into SBUF
            wqk_sb = []
            wv_sb = []
            wout_sb = []
            for kk in range(2):
                wqk_t = consts.tile([128, 512], dt.bfloat16, tag=f"wqk{kk}")
                nc.sync.dma_start(out=wqk_t, in_=wqk[kk * 128:(kk + 1) * 128, :])
                wqk_sb.append(wqk_t)
                wv_t = consts.tile([128, 256], dt.bfloat16, tag=f"wv{kk}")
                nc.sync.dma_start(out=wv_t, in_=wv[kk * 128:(kk + 1) * 128, :])
                wv_sb.append(wv_t)
                wout_t = consts.tile([128, 256], dt.bfloat16, tag=f"wout{kk}")
                nc.sync.dma_start(out=wout_t, in_=wout[kk * 128:(kk + 1) * 128, :])
                wout_sb.append(wout_t)
            eb_sb = consts.tile([128, 1024], dt.bfloat16, tag="eb")
            nc.sync.dma_start(out=eb_sb, in_=ebrep[:, 0:1024])
            id_sb = consts.tile([128, 128], dt.bfloat16, tag="id")
            nc.sync.dma_start(out=id_sb, in_=ident)
            bout_sb = consts.tile([128, 1, 256], dt.bfloat16, tag="bout")
            nc.sync.dma_start(out=bout_sb, in_=bout128)

            st = {}

            def load_x(s):
                img, sp = divmod(s, N_STRIPS)
                t0 = sp * STRIP
                xa = xp.tile([128, STRIP], dt.bfloat16, tag="xa")
                nc.sync.dma_start(out=xa, in_=xT[img, 0:128, t0:t0 + STRIP])
                xb = xp.tile([128, STRIP], dt.bfloat16, tag="xb")
                nc.sync.dma_start(out=xb, in_=xT[img, 128:256, t0:t0 + STRIP])
                st[s] = {"x": (xa, xb), "qk": [None] * 4, "edm": {},
                         "attn": [None] * NWP, "aot": [None] * 2,
                         "t0": t0, "img": img}

            def proj_qk_ot(s, ot):
                # ot: q(h0-3), q(h4-7), k(h0-3), k(h4-7)
                xa, xb = st[s]["x"]
                qkps = psp.tile([128, STRIP], dt.float32, tag="qkps",
                                bufs=QKPS_BUFS)
                nc.tensor.matmul(qkps, wqk_sb[0][:, ot * 128:(ot + 1) * 128],
                                 xa, start=True, stop=False)
                nc.tensor.matmul(qkps, wqk_sb[1][:, ot * 128:(ot + 1) * 128],
                                 xb, start=False, stop=True)
                qk_t = qkp.tile([128, STRIP], dt.bfloat16, tag="qk_t")
                copy_eng(QK_ENG[ot], qk_t, qkps)
                st[s]["qk"][ot] = qk_t

            def proj_v(s):
                # two window-pairs per PSUM tile; fused ones column for the
                # softmax denominator
                xa, xb = st[s]["x"]
                v_sb = []
                for vi in range(2):
                    vps = psp.tile([128, 2, NHEADS, CH], dt.float32,
                                   tag="small", bufs=SMALL_BUFS)
                    for half in range(2):
                        tb = 2 * vi + half
                        nc.tensor.matmul(vps[:, half, :, :],
                                         xa[:, tb * 128:(tb + 1) * 128],
                                         wv_sb[0], start=True, stop=False)
                        nc.tensor.matmul(vps[:, half, :, :],
                                         xb[:, tb * 128:(tb + 1) * 128],
                                         wv_sb[1], start=False, stop=True)
                    v3 = vp.tile([128, 2, NHEADS, CH + 1], dt.bfloat16,
                                 tag="v3")
                    nc.gpsimd.memset(v3[:, :, :, CH:CH + 1], 1.0)
                    copy_eng(V_ENG[vi], v3[:, :, :, 0:CH], vps)
                    v_sb.append(v3)
                st[s]["v"] = v_sb

            def dots_tiles(s, pairs):
                # flat [128, 1024] tiles (1-D free APs keep the DVE 2x uop
                # eligible); columns are (r2, wp) blocks of 128
                qk_sb = st[s]["qk"]
                for (hg, half) in pairs:
                    dps = psp.tile([128, 1024], dt.float32, tag="dps",
                                   bufs=DPS_BUFS)
                    for wp in range(NWP):
                        c0 = wp * 128
                        for r2 in range(2):
                            rg = 2 * half + r2
                            nc.tensor.matmul(
                                dps[:, r2 * 512 + c0:r2 * 512 + c0 + 128],
                                qk_sb[2 + hg][32 * rg:32 * rg + 32,
                                              c0:c0 + 128],
                                qk_sb[hg][32 * rg:32 * rg + 32,
                                          c0:c0 + 128],
                                start=True, stop=True,
                                tile_position=(32 * rg, 0),
                            )
                    ed = ep.tile([128, 1024], dt.bfloat16, tag="ed", bufs=6)
                    nc.scalar.activation(ed, dps, EXP)
                    edm = ep.tile([128, 1024], dt.bfloat16, tag="edm", bufs=12)
                    if EB_ENG[2 * hg + half] == "d":
                        nc.vector.tensor_mul(edm, ed, eb_sb)
                    else:
                        nc.gpsimd.tensor_mul(edm, ed, eb_sb)
                    st[s]["edm"][(hg, half)] = edm

            def att_av(s, wps):
                # AV matmuls per window-pair + normalize by the fused
                # denominator column
                edm_sb = st[s]["edm"]
                v_sb = st[s]["v"]
                for wp in wps:
                    c0 = wp * 128
                    avps = psp.tile([128, NHEADS, CH + 1], dt.float32,
                                    tag="small", bufs=SMALL_BUFS)
                    for hg in range(2):
                        for rg in range(4):
                            h = 4 * hg + rg
                            r2 = rg % 2
                            nc.tensor.matmul(
                                avps[:, h, :],
                                edm_sb[(hg, rg // 2)][:, r2 * 512 + c0:
                                                      r2 * 512 + c0 + 128],
                                v_sb[wp // 2][:, wp % 2, h, :],
                                start=True, stop=True,
                            )
                    attn = anp.tile([128, NHEADS, CH], dt.bfloat16,
                                    tag="attn")
                    if NORM_DIV:
                        nc.vector.tensor_tensor(
                            attn, avps[:, :, 0:CH],
                            avps[:, :, CH:CH + 1].to_broadcast(
                                (128, NHEADS, CH)), DIV)
                    else:
                        rd = anp.tile([128, NHEADS, 1], dt.float32, tag="rd")
                        nc.vector.reciprocal(rd, avps[:, :, CH:CH + 1])
                        nc.vector.tensor_mul(attn, avps[:, :, 0:CH],
                                             rd.to_broadcast(
                                                 (128, NHEADS, CH)))
                    st[s]["attn"][wp] = attn

            def att_transpose(s, vis):
                for vi in vis:
                    tps = psp.tile([128, 4, 128], dt.bfloat16,
                                   tag="small", bufs=SMALL_BUFS)
                    for half in range(2):
                        attn = st[s]["attn"][2 * vi + half]
                        for hh in range(2):
                            nc.tensor.transpose(
                                tps[:, 2 * half + hh, :],
                                attn[:, hh * 4:(hh + 1) * 4, :], id_sb)
                    aot = aotp.tile([128, 4, 128], dt.bfloat16, tag="aot")
                    if _env("AOT_ENG", "d") == "m":
                        nc.sync.dma_start(out=aot, in_=tps)
                    else:
                        nc.vector.tensor_copy(aot, tps)
                    st[s]["aot"][vi] = aot

            def att_out(s):
                t0 = st[s]["t0"]
                img = st[s]["img"]
                for vi in range(2):
                    aot = st[s]["aot"][vi]
                    ops = psp.tile([128, 2, 256], dt.float32, tag="small",
                                   bufs=SMALL_BUFS)
                    for half in range(2):
                        nc.tensor.matmul(ops[:, half, :],
                                         aot[:, 2 * half, :], wout_sb[0],
                                         start=True, stop=False)
                        nc.tensor.matmul(ops[:, half, :],
                                         aot[:, 2 * half + 1, :], wout_sb[1],
                                         start=False, stop=True)
                    fo = fop.tile([128, 2, 256], dt.bfloat16, tag="fo")
                    nc.vector.tensor_add(
                        fo, ops, bout_sb.to_broadcast((128, 2, 256)))
                    c0 = vi * 256
                    nc.sync.dma_start(
                        out=out[img, t0 + c0:t0 + c0 + 256, :].rearrange(
                            "(w p) c -> p w c", w=2),
                        in_=fo)
                del st[s]

            # Software pipeline: tail runs 2-3 slots behind the projections so
            # every cross-engine dependency has at least a slot of slack; the
            # tail itself is split in two chunks spread across the slot so the
            # small-PSUM rotation never gates back-to-back PE work.
            for s in range(NSLOT + 3):
                if s < NSLOT:
                    load_x(s)
                    proj_qk_ot(s, 0)
                if 2 <= s < NSLOT + 2:
                    att_av(s - 2, [0, 1, 2, 3])
                if s < NSLOT:
                    proj_qk_ot(s, 1)
                if 3 <= s:
                    att_out(s - 3)
                if s < NSLOT:
                    proj_qk_ot(s, 2)
                    proj_v(s)
                    dots_tiles(s, [(0, 0), (0, 1)])
                    proj_qk_ot(s, 3)
                if 2 <= s < NSLOT + 2:
                    att_transpose(s - 2, [0, 1])
                if s < NSLOT:
                    dots_tiles(s, [(1, 0), (1, 1)])
    nc.compile()
    return nc


def _host_prep(x, w_qkv, w_out, b_out, pos_embedding):
    ws = WS
    scale = CH ** -0.5
    xs = x.reshape(B * L, H // ws, ws, W // ws, ws, C)
    xs = xs.transpose(0, 1, 3, 2, 4, 5).reshape(IMG, T_IMG, C)
    xT = np.ascontiguousarray(xs.transpose(0, 2, 1)).astype(BF16)

    wq = (w_qkv[:, 0:256] * scale).astype(BF16)
    wk = w_qkv[:, 256:512].astype(BF16)
    wqk = np.concatenate([wq, wk], axis=1)
    wv = w_qkv[:, 512:768].astype(BF16)

    ri = _relative_indices(ws)
    bias = pos_embedding[ri[:, :, 0], ri[:, :, 1]]  # [i, j]
    ebT = np.exp(bias.astype(np.float64)).T.astype(np.float32)  # [j, i]
    ebsuper = np.zeros((128, 128), np.float32)
    ebsuper[0:64, 0:64] = ebT
    ebsuper[64:128, 64:128] = ebT
    ebrep = np.tile(ebsuper, (1, 16)).astype(BF16)

    ident = np.eye(128, dtype=BF16)
    bout128 = np.tile(b_out.reshape(1, C), (128, 1)).astype(BF16)

    return {
        "xT": xT,
        "wqk": np.ascontiguousarray(wqk),
        "wv": np.ascontiguousarray(wv),
        "wout": w_out.astype(BF16),
        "bout128": bout128,
        "ebrep": ebrep,
        "ident": ident,
    }


def kernel(x, w_qkv, w_out, b_out, pos_embedding, window_size, **extra):
    from concourse.bass_utils import run_bass_kernel_spmd

    x = np.asarray(x, dtype=np.float32)
    w_qkv = np.asarray(w_qkv, dtype=np.float32)
    w_out = np.asarray(w_out, dtype=np.float32)
    b_out = np.asarray(b_out, dtype=np.float32)
    pos_embedding = np.asarray(pos_embedding, dtype=np.float32)

    prep = _host_prep(x, w_qkv, w_out, b_out, pos_embedding)

    if "nc" not in _CACHE:
        _CACHE["nc"] = _build_kernel()
    nc = _CACHE["nc"]

    in_maps = []
    for core in range(N_CORES):
        m = dict(prep)
        m["xT"] = np.ascontiguousarray(
            prep["xT"][core * IMG_PER_CORE:(core + 1) * IMG_PER_CORE])
        in_maps.append(m)

    res = run_bass_kernel_spmd(nc, in_maps, core_ids=list(range(N_CORES)))
    outs = [res.results[c]["out"] for c in range(N_CORES)]
    o = np.concatenate(outs, axis=0)  # [16, 4096, 256]
    o = o.reshape(B * L, H // WS, W // WS, WS, WS, C)
    o = o.transpose(0, 1, 3, 2, 4, 5).reshape(B, L, H, W, C)
    return np.ascontiguousarray(o.astype(np.float32))
